# revision 10
# baseline (speedup 1.0000x reference)
"""Causal attention kernel for Trainium2 (8 NeuronCores, SPMD over heads).

Problem: B=4, H=16, S=2048, D=64, fp32.
  scores = Q @ K^T / sqrt(64); causal mask; softmax (global-max shift in the
  reference cancels exactly, so plain exp/rowsum is mathematically identical
  and numerically safe: |scores/8| <= ~7); out = attn @ V.

Distribution: B*H = 64 heads -> 8 heads per core, embarrassingly parallel.

Per-core algorithm (per head, two q-passes of 1024):
  - Host pre-transposes Q,K to [D,S] per head, so no on-device transposes.
  - scoresT[k,q] = sum_d K[k,d] Q[q,d] via f16 matmuls, k on partitions.
    Contraction is D=64, so even k-tiles use array rows 0-63 and odd k-tiles
    rows 64-127 (row packing -> 2 matmuls run concurrently).
  - exp is split between ScalarE (spline Exp, exact) and VectorE (Schraudolph
    2^x bit-trick into f16: at = bitcast<f16>(int16(s*A + B)), one
    tensor_scalar per tile) so both engines work the softmax concurrently.
  - Causal diag-block masking: zero the upper triangle of each diagonal
    128x128 block with a triangular f16 multiply on VectorE.
  - PV: out^T[m,q] = sum_k [V|ones]^T @ at accumulated in one PSUM tile with
    full 128-row contraction (no merge); row 64 is the softmax denominator.
  - Normalize in [m, q] layout (no PE transposes): round-trip the denominator
    row through DRAM to [128,8], reciprocal, round-trip back to a [1,1024]
    f16 row, broadcast it across partitions with a rank-1 matmul
    (ones65 x rrow) into the dead acc PSUM tile, then one tensor-tensor
    multiply. Output is stored as out^T [d, q]; the host transposes.
"""

import math
import os
import sys

import numpy as np

if "/opt/trn_rl_repo" not in sys.path:
    sys.path.insert(0, "/opt/trn_rl_repo")

B, H, S, D = 4, 16, 2048, 64
N_CORES = 8
HEADS_PER_CORE = (B * H) // N_CORES  # 8
PASS_Q = 1024  # q-columns per pass
CHUNK = 512  # PSUM-bank width in fp32 (matmul out may not cross banks)

# Schraudolph exp2 bit-trick constants for f16 output:
#   at = bitcast<f16>( int16( s * SCH_A + SCH_B ) ) ~= exp(s/8)
SCH_A = 1024.0 * math.log2(math.e) / 8.0  # 184.66496523378733
SCH_C = 47.0  # centering constant (tuned empirically)
SCH_B = 15360.0 - SCH_C

# Per-tile exp engine assignment, (pass, k) -> 'S' (ScalarE) | 'D' (VectorE).
ASSIGN = {
    (0, 0): "D", (0, 1): "S", (0, 2): "S", (0, 3): "D",
    (0, 4): "S", (0, 5): "D", (0, 6): "D", (0, 7): "D",
    (1, 0): "S", (1, 1): "D", (1, 2): "D", (1, 3): "D",
    (1, 4): "D", (1, 5): "S", (1, 6): "S", (1, 7): "S",
    (1, 8): "S", (1, 9): "S", (1, 10): "D", (1, 11): "D",
    (1, 12): "S", (1, 13): "S", (1, 14): "D", (1, 15): "D",
}


def _chunks(lo, hi):
    """Split [lo, hi) at absolute multiples of CHUNK (PSUM bank boundaries)."""
    out = []
    c = lo
    while c < hi:
        w = min(hi, (c // CHUNK + 1) * CHUNK) - c
        out.append((c, w))
        c += w
    return out


def build_attention(tc, outs, ins, scratch, n_heads=HEADS_PER_CORE, s=S, pass_q=PASS_Q):
    import concourse.bass as bass
    import concourse.mybir as mybir

    nc = tc.nc
    f32 = mybir.dt.float32
    f16 = mybir.dt.float16
    i16 = mybir.dt.int16
    Exp = mybir.ActivationFunctionType.Exp
    Mult = mybir.AluOpType.mult
    Add = mybir.AluOpType.add

    qt_d, kt_d, v_d = ins["qt"], ins["kt"], ins["v"]
    tri_d = ins["ctri"]
    drow_scr, rrow_scr = scratch["drow"], scratch["rrow"]
    ot_d = outs["ot"]

    n_ktiles = s // 128
    n_pass = s // pass_q
    n_qt = pass_q // 128

    with (
        tc.tile_pool(name="consts", bufs=1) as cpool,
        tc.tile_pool(name="qpool", bufs=2) as qpool,
        tc.tile_pool(name="kpool", bufs=2) as kpool,
        tc.tile_pool(name="vpool", bufs=2) as vpool,
        tc.tile_pool(name="atpool", bufs=7) as atpool,
        tc.tile_pool(name="osbpool", bufs=2) as osbpool,
        tc.tile_pool(name="ofpool", bufs=2) as ofpool,
        tc.tile_pool(name="nrmpool", bufs=2) as nrmpool,
        tc.tile_pool(name="scpool", bufs=3, space="PSUM") as scpool,
        tc.tile_pool(name="accpool", bufs=1, space="PSUM") as accpool,
    ):
        c_tri = cpool.tile([128, 128], f16, tag="ctri")
        nc.sync.dma_start(c_tri[:], tri_d[:])
        ones65 = cpool.tile([1, 65], f16, tag="ones65")
        nc.vector.memset(ones65[:], 1.0)

        pending_norm = [None]

        def _flush_norm():
            if pending_norm[0] is not None:
                pending_norm[0]()
                pending_norm[0] = None

        def load_head(h):
            # Q^T duplicated into both partition halves (for row packing).
            qt2 = qpool.tile([128, s], f16, name=f"qt2_{h}")
            nc.sync.dma_start(qt2[0:64, :], qt_d[h])
            nc.sync.dma_start(qt2[64:128, :], qt_d[h])
            # K^T: even k-tiles -> partitions 0-63, odd -> 64-127.
            kt2 = kpool.tile([128, s // 2], f16, name=f"kt2_{h}")
            kt_src = kt_d[h].rearrange("d (t two c) -> d two t c", two=2, c=128)
            kt2_v = kt2.rearrange("p (t c) -> p t c", c=128)
            nc.sync.dma_start(kt2_v[0:64], kt_src[:, 0])
            nc.sync.dma_start(kt2_v[64:128], kt_src[:, 1])
            # V with a ones-column pre-appended on the host: [128, n_ktiles, 65].
            vx = vpool.tile([128, n_ktiles * 65], f16, name=f"vx_{h}")
            vx_v = vx.rearrange("p (t c) -> p t c", c=65)
            nc.sync.dma_start(
                vx_v[:], v_d[h].rearrange("(t p) d -> p t d", p=128)
            )
            return qt2, kt2_v, vx_v

        head_tiles = load_head(0)

        for h in range(n_heads):
            qt2, kt2_v, vx_v = head_tiles
            next_tiles = [None]

            for p in range(n_pass):
                if p == 1 and h + 1 < n_heads:
                    # prefetch next head's inputs ahead of this head's
                    # epilogue DMAs so QK never stalls at the head boundary
                    next_tiles[0] = load_head(h + 1)
                q0 = p * pass_q
                kmax = (p + 1) * (pass_q // 128)
                acc = accpool.tile([65, pass_q], f32, name=f"acc_{h}_{p}", tag="acc")
                pv_queue = []

                def _emit_pv(entries, acc=acc, q0=q0, kmax=kmax, vx_v=vx_v):
                    for (k, at, qlo) in entries:
                        for (c, w) in _chunks(qlo - q0, pass_q):
                            co = c - (qlo - q0)
                            nc.tensor.matmul(
                                acc[0:65, c : c + w],
                                vx_v[:, k, :],
                                at[:, co : co + w],
                                start=(k == 0),
                                stop=(k == kmax - 1),
                                skip_group_check=True,
                            )

                for kp in range(0, kmax, 2):
                    pair = [k for k in (kp, kp + 1) if k < kmax]
                    scs, spans, qlos = {}, {}, {}
                    for k in pair:
                        qlos[k] = max(q0, 128 * k)
                        spans[k] = q0 + pass_q - qlos[k]
                        scs[k] = scpool.tile(
                            [128, pass_q], f32, tag="sc", name=f"sc_{h}_{p}_{k}"
                        )
                    # interleave even/odd chunks so the two row-groups of the
                    # PE array (d=64 contraction) run concurrently
                    chunk_lists = {k: _chunks(0, spans[k]) for k in pair}
                    n_ch = max(len(v) for v in chunk_lists.values())
                    for ci in range(n_ch):
                        for k in pair:
                            if ci >= len(chunk_lists[k]):
                                continue
                            c, w = chunk_lists[k][ci]
                            half = k % 2
                            nc.tensor.matmul(
                                scs[k][:, c : c + w],
                                kt2_v[64 * half : 64 * half + 64, k // 2],
                                qt2[64 * half : 64 * half + 64, qlos[k] + c : qlos[k] + c + w],
                                start=True,
                                stop=True,
                                skip_group_check=True,
                            )
                    if kp >= 2:
                        _flush_norm()
                    cur = []
                    for k in pair:
                        span = spans[k]
                        at = atpool.tile([128, pass_q], f16)
                        if ASSIGN[(p, k)] == "S":
                            nc.scalar.activation(
                                at[:, 0:span], scs[k][:, 0:span], Exp, scale=0.125
                            )
                        else:
                            nc.vector.tensor_scalar(
                                at[:, 0:span].bitcast(i16),
                                scs[k][:, 0:span],
                                SCH_A,
                                SCH_B,
                                Mult,
                                Add,
                            )
                        if 128 * k >= q0:
                            # zero the masked upper part of the diagonal block
                            nc.vector.tensor_mul(at[:, 0:128], at[:, 0:128], c_tri[:])
                        cur.append((k, at, qlos[k]))
                    pv_queue.append(cur)
                    if len(pv_queue) > 2:
                        _emit_pv(pv_queue.pop(0))
                for entries in pv_queue:
                    _emit_pv(entries)

                # evacuate PSUM early (split between ScalarE and VectorE) so
                # acc can be reused; the rest of the epilogue is deferred so
                # the next pass's QK matmuls hide it.
                osb = osbpool.tile([65, pass_q], f32, name=f"osb_{h}_{q0}", tag="osb")
                nc.scalar.copy(osb[:, 0:512], acc[0:65, 0:512])
                nc.vector.tensor_copy(osb[:, 512:1024], acc[0:65, 512:1024])

                def _norm(osb=osb, acc=acc, h=h, p=p, q0=q0):
                    # denominator row -> [128, 8] via DRAM round-trip (the
                    # DMA engines do the cross-partition reshape, not the PE)
                    drow = nrmpool.tile([128, n_qt], f32, tag="drow", name=f"drow_{h}_{q0}")
                    nc.gpsimd.dma_start(drow_scr[p], osb[64:65, :])
                    nc.gpsimd.dma_start(
                        drow[:], drow_scr[p].rearrange("(t p) -> p t", p=128)
                    )
                    rcp = nrmpool.tile([128, n_qt], f32, tag="rcp", name=f"rcp_{h}_{q0}")
                    rsc = nrmpool.tile([128, n_qt], f32, tag="rsc", name=f"rsc_{h}_{q0}")
                    nc.vector.reciprocal_approx_accurate(rcp[:], drow[:], rsc[:])
                    rcp16 = nrmpool.tile([128, n_qt], f16, tag="rcp16", name=f"rcp16_{h}_{q0}")
                    nc.vector.tensor_copy(rcp16[:], rcp[:])
                    rrow = nrmpool.tile([1, pass_q], f16, tag="rrow", name=f"rrow_{h}_{q0}")
                    nc.gpsimd.dma_start(
                        rrow_scr[p].rearrange("(t p) -> p t", p=128), rcp16[:]
                    )
                    nc.gpsimd.dma_start(rrow[:], rrow_scr[p])
                    # broadcast 1/denom across partitions: rank-1 matmul into
                    # the (dead) acc tile, then one TT multiply normalizes.
                    for c in (0, 512):
                        nc.tensor.matmul(
                            acc[0:65, c : c + 512],
                            ones65[:],
                            rrow[:, c : c + 512],
                            start=True,
                            stop=True,
                            skip_group_check=True,
                        )
                    of = ofpool.tile([64, pass_q], f32, tag="of", name=f"of_{h}_{q0}")
                    nc.vector.tensor_mul(of[:], osb[0:64, :], acc[0:64, :])
                    nc.sync.dma_start(ot_d[h, :, q0 : q0 + pass_q], of[:])

                pending_norm[0] = _norm
            _flush_norm()
            head_tiles = next_tiles[0]


def _make_consts():
    kk, qq = np.meshgrid(np.arange(128), np.arange(128), indexing="ij")
    tri = (kk <= qq).astype(np.float16)  # keep-mask for the diagonal block
    return (tri,)


_NC_CACHE = {}


def _build_nc(n_heads=HEADS_PER_CORE, s=S, pass_q=PASS_Q):
    key = (n_heads, s, pass_q)
    if key in _NC_CACHE:
        return _NC_CACHE[key]
    import concourse.tile as tile
    from concourse import bacc, mybir

    nc = bacc.Bacc(
        "TRN2", target_bir_lowering=False, debug=False, enable_asserts=False
    )
    f32 = mybir.dt.float32
    f16 = mybir.dt.float16
    ins = {
        "qt": nc.dram_tensor("qt", [n_heads, D, s], f16, kind="ExternalInput").ap(),
        "kt": nc.dram_tensor("kt", [n_heads, D, s], f16, kind="ExternalInput").ap(),
        "v": nc.dram_tensor("v", [n_heads, s, D + 1], f16, kind="ExternalInput").ap(),
        "ctri": nc.dram_tensor("ctri", [128, 128], f16, kind="ExternalInput").ap(),
    }
    scratch = {
        "drow": nc.dram_tensor("drow_scr", [2, pass_q], f32, kind="Internal").ap(),
        "rrow": nc.dram_tensor("rrow_scr", [2, pass_q], f16, kind="Internal").ap(),
    }
    outs = {
        # out^T per head: [d, q]; the host transposes back to [q, d]
        "ot": nc.dram_tensor("ot", [n_heads, D, s], f32, kind="ExternalOutput").ap(),
    }
    with tile.TileContext(nc) as tc:
        build_attention(tc, outs, ins, scratch, n_heads=n_heads, s=s, pass_q=pass_q)
    nc.compile()
    _NC_CACHE[key] = nc
    return nc


def kernel(Q, K, V, mask, trace=False):
    """Full-input entry point: shards over 8 NeuronCores, returns full output."""
    from concourse.bass_utils import run_bass_kernel_spmd

    nc = _build_nc()
    (tri,) = _make_consts()

    Qf = np.ascontiguousarray(
        Q.reshape(B * H, S, D).transpose(0, 2, 1), dtype=np.float16
    )
    Kf = np.ascontiguousarray(
        K.reshape(B * H, S, D).transpose(0, 2, 1), dtype=np.float16
    )
    Vf = np.concatenate(
        [
            V.reshape(B * H, S, D).astype(np.float16),
            np.ones((B * H, S, 1), dtype=np.float16),
        ],
        axis=-1,
    )

    in_maps = []
    for c in range(N_CORES):
        sl = slice(c * HEADS_PER_CORE, (c + 1) * HEADS_PER_CORE)
        in_maps.append(
            {
                "qt": Qf[sl],
                "kt": Kf[sl],
                "v": Vf[sl],
                "ctri": tri,
            }
        )

    res = run_bass_kernel_spmd(nc, in_maps, core_ids=list(range(N_CORES)), trace=trace)
    ot = np.concatenate([res.results[c]["ot"] for c in range(N_CORES)], axis=0)
    out = ot.transpose(0, 2, 1).reshape(B, H, S, D)
    kernel.last_results = res
    return np.ascontiguousarray(out, dtype=np.float32)


# revision 11
# speedup vs baseline: 1.3209x; 1.3209x over previous
"""Causal attention kernel for Trainium2 (8 NeuronCores, SPMD over heads).

Problem: B=4, H=16, S=2048, D=64, fp32.
  scores = Q @ K^T / sqrt(64); causal mask; softmax (global-max shift in the
  reference cancels exactly, so plain exp/rowsum is mathematically identical
  and numerically safe: |scores/8| <= ~7); out = attn @ V.

Distribution: B*H = 64 heads -> 8 heads per core, embarrassingly parallel.

Per-core algorithm (per head, two q-passes of 1024):
  - Host pre-transposes Q,K to [D,S] per head, so no on-device transposes.
  - scoresT[k,q] = sum_d K[k,d] Q[q,d] via f16 matmuls, k on partitions.
    Contraction is D=64, so even k-tiles use array rows 0-63 and odd k-tiles
    rows 64-127 (row packing -> 2 matmuls run concurrently).
  - exp is split between ScalarE (spline Exp, exact) and VectorE (Schraudolph
    2^x bit-trick into f16: at = bitcast<f16>(int16(s*A + B)), one
    tensor_scalar per tile) so both engines work the softmax concurrently.
  - Causal diag-block masking: zero the upper triangle of each diagonal
    128x128 block with a triangular f16 multiply on VectorE.
  - PV: out^T[m,q] = sum_k [V|ones]^T @ at accumulated in one PSUM tile with
    full 128-row contraction (no merge); row 64 is the softmax denominator.
  - Normalize in [m, q] layout (no PE transposes): round-trip the denominator
    row through DRAM to [128,8], reciprocal, round-trip back to a [1,1024]
    f16 row, broadcast it across partitions with a rank-1 matmul
    (ones65 x rrow) into the dead acc PSUM tile, then one tensor-tensor
    multiply. Output is stored as out^T [d, q]; the host transposes.
"""

import math
import os
import sys

import numpy as np

if "/opt/trn_rl_repo" not in sys.path:
    sys.path.insert(0, "/opt/trn_rl_repo")

B, H, S, D = 4, 16, 2048, 64
N_CORES = 8
HEADS_PER_CORE = (B * H) // N_CORES  # 8
PASS_Q = 1024  # q-columns per pass
CHUNK = 512  # PSUM-bank width in fp32 (matmul out may not cross banks)

# Schraudolph exp2 bit-trick constants for f16 output:
#   at = bitcast<f16>( int16( s * SCH_A + SCH_B ) ) ~= exp(s/8)
SCH_A = 1024.0 * math.log2(math.e) / 8.0  # 184.66496523378733
SCH_C = 47.0  # centering constant (tuned empirically)
SCH_B = 15360.0 - SCH_C

# Per-tile exp engine assignment, (pass, k) -> 'S' (ScalarE) | 'D' (VectorE).
ASSIGN = {
    (0, 0): "D", (0, 1): "S", (0, 2): "S", (0, 3): "D",
    (0, 4): "S", (0, 5): "D", (0, 6): "D", (0, 7): "D",
    (1, 0): "S", (1, 1): "D", (1, 2): "D", (1, 3): "D",
    (1, 4): "D", (1, 5): "S", (1, 6): "S", (1, 7): "S",
    (1, 8): "S", (1, 9): "S", (1, 10): "D", (1, 11): "D",
    (1, 12): "S", (1, 13): "S", (1, 14): "D", (1, 15): "D",
}


def _chunks(lo, hi):
    """Split [lo, hi) at absolute multiples of CHUNK (PSUM bank boundaries)."""
    out = []
    c = lo
    while c < hi:
        w = min(hi, (c // CHUNK + 1) * CHUNK) - c
        out.append((c, w))
        c += w
    return out


def build_attention(tc, outs, ins, scratch, n_heads=HEADS_PER_CORE, s=S, pass_q=PASS_Q):
    import concourse.bass as bass
    import concourse.mybir as mybir

    nc = tc.nc
    f32 = mybir.dt.float32
    f16 = mybir.dt.float16
    i16 = mybir.dt.int16
    Exp = mybir.ActivationFunctionType.Exp
    Mult = mybir.AluOpType.mult
    Add = mybir.AluOpType.add

    qt_d, kt_d, v_d = ins["qt"], ins["kt"], ins["v"]
    tri_d = ins["ctri"]
    drow_scr, rrow_scr = scratch["drow"], scratch["rrow"]
    ot_d = outs["ot"]

    n_ktiles = s // 128
    n_pass = s // pass_q
    n_qt = pass_q // 128

    with (
        tc.tile_pool(name="consts", bufs=1) as cpool,
        tc.tile_pool(name="qpool", bufs=2) as qpool,
        tc.tile_pool(name="kpool", bufs=2) as kpool,
        tc.tile_pool(name="vpool", bufs=2) as vpool,
        tc.tile_pool(name="atpool", bufs=7) as atpool,
        tc.tile_pool(name="osbpool", bufs=2) as osbpool,
        tc.tile_pool(name="ofpool", bufs=2) as ofpool,
        tc.tile_pool(name="nrmpool", bufs=2) as nrmpool,
        tc.tile_pool(name="scpool", bufs=2, space="PSUM") as scpool,
        tc.tile_pool(name="accpool", bufs=1, space="PSUM") as accpool,
        tc.tile_pool(name="rcpbpool", bufs=1, space="PSUM") as rcpbpool,
    ):
        c_tri = cpool.tile([128, 128], f16, tag="ctri")
        nc.sync.dma_start(c_tri[:], tri_d[:])
        ones65 = cpool.tile([1, 65], f16, tag="ones65")
        nc.vector.memset(ones65[:], 1.0)

        pending_norm = [None]

        def _flush_norm():
            if pending_norm[0] is not None:
                pending_norm[0]()
                pending_norm[0] = None

        def load_head(h):
            # Q^T duplicated into both partition halves (for row packing).
            qt2 = qpool.tile([128, s], f16, name=f"qt2_{h}")
            nc.sync.dma_start(qt2[0:64, :], qt_d[h])
            nc.sync.dma_start(qt2[64:128, :], qt_d[h])
            # K^T: even k-tiles -> partitions 0-63, odd -> 64-127.
            kt2 = kpool.tile([128, s // 2], f16, name=f"kt2_{h}")
            kt_src = kt_d[h].rearrange("d (t two c) -> d two t c", two=2, c=128)
            kt2_v = kt2.rearrange("p (t c) -> p t c", c=128)
            nc.sync.dma_start(kt2_v[0:64], kt_src[:, 0])
            nc.sync.dma_start(kt2_v[64:128], kt_src[:, 1])
            # V with a ones-column pre-appended on the host: [128, n_ktiles, 65].
            vx = vpool.tile([128, n_ktiles * 65], f16, name=f"vx_{h}")
            vx_v = vx.rearrange("p (t c) -> p t c", c=65)
            nc.sync.dma_start(
                vx_v[:], v_d[h].rearrange("(t p) d -> p t d", p=128)
            )
            return qt2, kt2_v, vx_v

        head_tiles = load_head(0)

        for h in range(n_heads):
            qt2, kt2_v, vx_v = head_tiles
            next_tiles = [None]

            for p in range(n_pass):
                if p == 1 and h + 1 < n_heads:
                    # prefetch next head's inputs ahead of this head's
                    # epilogue DMAs so QK never stalls at the head boundary
                    next_tiles[0] = load_head(h + 1)
                q0 = p * pass_q
                kmax = (p + 1) * (pass_q // 128)
                acc = accpool.tile([65, pass_q], f32, name=f"acc_{h}_{p}", tag="acc")
                pv_queue = []

                def _emit_pv(entries, acc=acc, q0=q0, kmax=kmax, vx_v=vx_v):
                    for (k, at, qlo) in entries:
                        for (c, w) in _chunks(qlo - q0, pass_q):
                            co = c - (qlo - q0)
                            nc.tensor.matmul(
                                acc[0:65, c : c + w],
                                vx_v[:, k, :],
                                at[:, co : co + w],
                                start=(k == 0),
                                stop=(k == kmax - 1),
                                skip_group_check=True,
                            )

                for kp in range(0, kmax, 2):
                    pair = [k for k in (kp, kp + 1) if k < kmax]
                    scs, spans, qlos = {}, {}, {}
                    for k in pair:
                        qlos[k] = max(q0, 128 * k)
                        spans[k] = q0 + pass_q - qlos[k]
                        scs[k] = scpool.tile(
                            [128, pass_q], f32, tag="sc", name=f"sc_{h}_{p}_{k}"
                        )
                    # interleave even/odd chunks so the two row-groups of the
                    # PE array (d=64 contraction) run concurrently
                    chunk_lists = {k: _chunks(0, spans[k]) for k in pair}
                    n_ch = max(len(v) for v in chunk_lists.values())
                    for ci in range(n_ch):
                        for k in pair:
                            if ci >= len(chunk_lists[k]):
                                continue
                            c, w = chunk_lists[k][ci]
                            half = k % 2
                            nc.tensor.matmul(
                                scs[k][:, c : c + w],
                                kt2_v[64 * half : 64 * half + 64, k // 2],
                                qt2[64 * half : 64 * half + 64, qlos[k] + c : qlos[k] + c + w],
                                start=True,
                                stop=True,
                                skip_group_check=True,
                            )
                    if kp >= 2:
                        _flush_norm()
                    cur = []
                    for k in pair:
                        span = spans[k]
                        at = atpool.tile([128, pass_q], f16)
                        if ASSIGN[(p, k)] == "S":
                            nc.scalar.activation(
                                at[:, 0:span], scs[k][:, 0:span], Exp, scale=0.125
                            )
                        else:
                            nc.vector.tensor_scalar(
                                at[:, 0:span].bitcast(i16),
                                scs[k][:, 0:span],
                                SCH_A,
                                SCH_B,
                                Mult,
                                Add,
                            )
                        if 128 * k >= q0:
                            # zero the masked upper part of the diagonal block
                            nc.vector.tensor_mul(at[:, 0:128], at[:, 0:128], c_tri[:])
                        cur.append((k, at, qlos[k]))
                    pv_queue.append(cur)
                    if len(pv_queue) > 2:
                        _emit_pv(pv_queue.pop(0))
                for entries in pv_queue:
                    _emit_pv(entries)

                # evacuate PSUM early (split between ScalarE and VectorE) so
                # acc can be reused; the rest of the epilogue is deferred so
                # the next pass's QK matmuls hide it.
                osb = osbpool.tile([65, pass_q], f32, name=f"osb_{h}_{q0}", tag="osb")
                nc.scalar.copy(osb[:, 0:512], acc[0:65, 0:512])
                nc.vector.tensor_copy(osb[:, 512:1024], acc[0:65, 512:1024])

                def _norm(osb=osb, h=h, p=p, q0=q0):
                    # denominator row -> [128, 8] via DRAM round-trip (the
                    # DMA engines do the cross-partition reshape, not the PE)
                    drow = nrmpool.tile([128, n_qt], f32, tag="drow", name=f"drow_{h}_{q0}")
                    nc.sync.dma_start(drow_scr[p], osb[64:65, :])
                    nc.sync.dma_start(
                        drow[:], drow_scr[p].rearrange("(t p) -> p t", p=128)
                    )
                    rcp = nrmpool.tile([128, n_qt], f32, tag="rcp", name=f"rcp_{h}_{q0}")
                    rsc = nrmpool.tile([128, n_qt], f32, tag="rsc", name=f"rsc_{h}_{q0}")
                    nc.vector.reciprocal_approx_accurate(rcp[:], drow[:], rsc[:])
                    rcp16 = nrmpool.tile([128, n_qt], f16, tag="rcp16", name=f"rcp16_{h}_{q0}")
                    nc.vector.tensor_copy(rcp16[:], rcp[:])
                    rrow = nrmpool.tile([1, pass_q], f16, tag="rrow", name=f"rrow_{h}_{q0}")
                    nc.sync.dma_start(
                        rrow_scr[p].rearrange("(t p) -> p t", p=128), rcp16[:]
                    )
                    nc.sync.dma_start(rrow[:], rrow_scr[p])
                    # broadcast 1/denom across partitions: rank-1 matmul into
                    # the (dead) acc tile, then one TT multiply normalizes.
                    rcpb = rcpbpool.tile([65, pass_q], f32, tag="rcpb", name=f"rcpb_{h}_{q0}")
                    for c in (0, 512):
                        nc.tensor.matmul(
                            rcpb[0:65, c : c + 512],
                            ones65[:],
                            rrow[:, c : c + 512],
                            start=True,
                            stop=True,
                            skip_group_check=True,
                        )
                    of = ofpool.tile([64, pass_q], f32, tag="of", name=f"of_{h}_{q0}")
                    nc.vector.tensor_mul(of[:], osb[0:64, :], rcpb[0:64, :])
                    nc.sync.dma_start(ot_d[h, :, q0 : q0 + pass_q], of[:])

                pending_norm[0] = _norm
            _flush_norm()
            head_tiles = next_tiles[0]


def _make_consts():
    kk, qq = np.meshgrid(np.arange(128), np.arange(128), indexing="ij")
    tri = (kk <= qq).astype(np.float16)  # keep-mask for the diagonal block
    return (tri,)


_NC_CACHE = {}


def _build_nc(n_heads=HEADS_PER_CORE, s=S, pass_q=PASS_Q):
    key = (n_heads, s, pass_q)
    if key in _NC_CACHE:
        return _NC_CACHE[key]
    import concourse.tile as tile
    from concourse import bacc, mybir

    nc = bacc.Bacc(
        "TRN2", target_bir_lowering=False, debug=False, enable_asserts=False
    )
    f32 = mybir.dt.float32
    f16 = mybir.dt.float16
    ins = {
        "qt": nc.dram_tensor("qt", [n_heads, D, s], f16, kind="ExternalInput").ap(),
        "kt": nc.dram_tensor("kt", [n_heads, D, s], f16, kind="ExternalInput").ap(),
        "v": nc.dram_tensor("v", [n_heads, s, D + 1], f16, kind="ExternalInput").ap(),
        "ctri": nc.dram_tensor("ctri", [128, 128], f16, kind="ExternalInput").ap(),
    }
    scratch = {
        "drow": nc.dram_tensor("drow_scr", [2, pass_q], f32, kind="Internal").ap(),
        "rrow": nc.dram_tensor("rrow_scr", [2, pass_q], f16, kind="Internal").ap(),
    }
    outs = {
        # out^T per head: [d, q]; the host transposes back to [q, d]
        "ot": nc.dram_tensor("ot", [n_heads, D, s], f32, kind="ExternalOutput").ap(),
    }
    with tile.TileContext(nc) as tc:
        build_attention(tc, outs, ins, scratch, n_heads=n_heads, s=s, pass_q=pass_q)
    nc.compile()
    _NC_CACHE[key] = nc
    return nc


def kernel(Q, K, V, mask, trace=False):
    """Full-input entry point: shards over 8 NeuronCores, returns full output."""
    from concourse.bass_utils import run_bass_kernel_spmd

    nc = _build_nc()
    (tri,) = _make_consts()

    Qf = np.ascontiguousarray(
        Q.reshape(B * H, S, D).transpose(0, 2, 1), dtype=np.float16
    )
    Kf = np.ascontiguousarray(
        K.reshape(B * H, S, D).transpose(0, 2, 1), dtype=np.float16
    )
    Vf = np.concatenate(
        [
            V.reshape(B * H, S, D).astype(np.float16),
            np.ones((B * H, S, 1), dtype=np.float16),
        ],
        axis=-1,
    )

    in_maps = []
    for c in range(N_CORES):
        sl = slice(c * HEADS_PER_CORE, (c + 1) * HEADS_PER_CORE)
        in_maps.append(
            {
                "qt": Qf[sl],
                "kt": Kf[sl],
                "v": Vf[sl],
                "ctri": tri,
            }
        )

    res = run_bass_kernel_spmd(nc, in_maps, core_ids=list(range(N_CORES)), trace=trace)
    ot = np.concatenate([res.results[c]["ot"] for c in range(N_CORES)], axis=0)
    out = ot.transpose(0, 2, 1).reshape(B, H, S, D)
    kernel.last_results = res
    return np.ascontiguousarray(out, dtype=np.float32)


# revision 12
# speedup vs baseline: 2.9259x; 2.2151x over previous
"""Causal attention kernel for Trainium2 (8 NeuronCores, SPMD over heads).

Problem: B=4, H=16, S=2048, D=64, fp32.
  scores = Q @ K^T / sqrt(64); causal mask; softmax (global-max shift in the
  reference cancels exactly, so plain exp/rowsum is mathematically identical
  and numerically safe: |scores/8| <= ~7); out = attn @ V.

Distribution: B*H = 64 heads -> 8 heads per core, embarrassingly parallel.

Per-core algorithm (per head, two q-passes of 1024):
  - Host pre-transposes Q,K to [D,S] per head, so no on-device transposes.
  - scoresT[k,q] = sum_d K[k,d] Q[q,d] via f16 matmuls, k on partitions.
    Contraction is D=64, so even k-tiles use array rows 0-63 and odd k-tiles
    rows 64-127 (row packing -> 2 matmuls run concurrently).
  - exp is split between ScalarE (spline Exp, exact) and VectorE (Schraudolph
    2^x bit-trick into f16: at = bitcast<f16>(int16(s*A + B)), one
    tensor_scalar per tile) so both engines work the softmax concurrently.
  - Causal diag-block masking: zero the upper triangle of each diagonal
    128x128 block with a triangular f16 multiply on VectorE.
  - PV: out^T[m,q] = sum_k [V|ones]^T @ at accumulated in one PSUM tile with
    full 128-row contraction (no merge); row 64 is the softmax denominator.
  - Normalize: PE-transpose out^T to [q, 65] (col 64 = rowsum), reciprocal
    per partition, one broadcast multiply, DMA out. Next head's inputs are
    prefetched before this epilogue so QK never stalls on the Sync queue.
"""

import math
import os
import sys

import numpy as np

if "/opt/trn_rl_repo" not in sys.path:
    sys.path.insert(0, "/opt/trn_rl_repo")

B, H, S, D = 4, 16, 2048, 64
N_CORES = 8
HEADS_PER_CORE = (B * H) // N_CORES  # 8
PASS_Q = 1024  # q-columns per pass
CHUNK = 512  # PSUM-bank width in fp32 (matmul out may not cross banks)

# Schraudolph exp2 bit-trick constants for f16 output:
#   at = bitcast<f16>( int16( s * SCH_A + SCH_B ) ) ~= exp(s/8)
SCH_A = 1024.0 * math.log2(math.e) / 8.0  # 184.66496523378733
SCH_C = 47.0  # centering constant (tuned empirically)
SCH_B = 15360.0 - SCH_C

# Per-tile exp engine assignment, (pass, k) -> 'S' (ScalarE) | 'D' (VectorE).
ASSIGN = {
    (0, 0): "D", (0, 1): "S", (0, 2): "S", (0, 3): "D",
    (0, 4): "S", (0, 5): "D", (0, 6): "D", (0, 7): "D",
    (1, 0): "S", (1, 1): "D", (1, 2): "D", (1, 3): "D",
    (1, 4): "D", (1, 5): "S", (1, 6): "S", (1, 7): "S",
    (1, 8): "S", (1, 9): "S", (1, 10): "D", (1, 11): "D",
    (1, 12): "S", (1, 13): "S", (1, 14): "D", (1, 15): "D",
}


def _chunks(lo, hi):
    """Split [lo, hi) at absolute multiples of CHUNK (PSUM bank boundaries)."""
    out = []
    c = lo
    while c < hi:
        w = min(hi, (c // CHUNK + 1) * CHUNK) - c
        out.append((c, w))
        c += w
    return out


def build_attention(tc, outs, ins, n_heads=HEADS_PER_CORE, s=S, pass_q=PASS_Q):
    import concourse.bass as bass
    import concourse.mybir as mybir

    nc = tc.nc
    f32 = mybir.dt.float32
    f16 = mybir.dt.float16
    i16 = mybir.dt.int16
    Exp = mybir.ActivationFunctionType.Exp
    Mult = mybir.AluOpType.mult
    Add = mybir.AluOpType.add

    qt_d, kt_d, v_d = ins["qt"], ins["kt"], ins["v"]
    tri_d = ins["ctri"]
    iden65_d = ins["ciden65"]
    ot_d = outs["ot"]

    n_ktiles = s // 128
    n_pass = s // pass_q
    n_qt = pass_q // 128

    with (
        tc.tile_pool(name="consts", bufs=1) as cpool,
        tc.tile_pool(name="qpool", bufs=2) as qpool,
        tc.tile_pool(name="kpool", bufs=2) as kpool,
        tc.tile_pool(name="vpool", bufs=2) as vpool,
        tc.tile_pool(name="atpool", bufs=7) as atpool,
        tc.tile_pool(name="osbpool", bufs=2) as osbpool,
        tc.tile_pool(name="ofpool", bufs=2) as ofpool,
        tc.tile_pool(name="nrmpool", bufs=2) as nrmpool,
        tc.tile_pool(name="scpool", bufs=2, space="PSUM") as scpool,
        tc.tile_pool(name="accpool", bufs=1, space="PSUM") as accpool,
        tc.tile_pool(name="trpool", bufs=1, space="PSUM") as trpool,
    ):
        c_tri = cpool.tile([128, 128], f16, tag="ctri")
        nc.sync.dma_start(c_tri[:], tri_d[:])
        iden65 = cpool.tile([65, 65], f32, tag="iden65")
        nc.sync.dma_start(iden65[:], iden65_d[:])

        pending_norm = [None]

        def _flush_norm():
            if pending_norm[0] is not None:
                pending_norm[0]()
                pending_norm[0] = None

        def load_head(h):
            # Q^T duplicated into both partition halves (for row packing).
            qt2 = qpool.tile([128, s], f16, name=f"qt2_{h}")
            nc.sync.dma_start(qt2[0:64, :], qt_d[h])
            nc.sync.dma_start(qt2[64:128, :], qt_d[h])
            # K^T: even k-tiles -> partitions 0-63, odd -> 64-127.
            kt2 = kpool.tile([128, s // 2], f16, name=f"kt2_{h}")
            kt_src = kt_d[h].rearrange("d (t two c) -> d two t c", two=2, c=128)
            kt2_v = kt2.rearrange("p (t c) -> p t c", c=128)
            nc.sync.dma_start(kt2_v[0:64], kt_src[:, 0])
            nc.sync.dma_start(kt2_v[64:128], kt_src[:, 1])
            # V with a ones-column pre-appended on the host: [128, n_ktiles, 65].
            vx = vpool.tile([128, n_ktiles * 65], f16, name=f"vx_{h}")
            vx_v = vx.rearrange("p (t c) -> p t c", c=65)
            nc.sync.dma_start(
                vx_v[:], v_d[h].rearrange("(t p) d -> p t d", p=128)
            )
            return qt2, kt2_v, vx_v

        head_tiles = load_head(0)

        for h in range(n_heads):
            qt2, kt2_v, vx_v = head_tiles
            next_tiles = [None]

            for p in range(n_pass):
                if p == 1 and h + 1 < n_heads:
                    # prefetch next head's inputs ahead of this head's
                    # epilogue DMAs so QK never stalls at the head boundary
                    next_tiles[0] = load_head(h + 1)
                q0 = p * pass_q
                kmax = (p + 1) * (pass_q // 128)
                acc = accpool.tile([65, pass_q], f32, name=f"acc_{h}_{p}", tag="acc")
                pv_queue = []

                def _emit_pv(entries, acc=acc, q0=q0, kmax=kmax, vx_v=vx_v):
                    for (k, at, qlo) in entries:
                        for (c, w) in _chunks(qlo - q0, pass_q):
                            co = c - (qlo - q0)
                            nc.tensor.matmul(
                                acc[0:65, c : c + w],
                                vx_v[:, k, :],
                                at[:, co : co + w],
                                start=(k == 0),
                                stop=(k == kmax - 1),
                                skip_group_check=True,
                            )

                for kp in range(0, kmax, 2):
                    pair = [k for k in (kp, kp + 1) if k < kmax]
                    scs, spans, qlos = {}, {}, {}
                    for k in pair:
                        qlos[k] = max(q0, 128 * k)
                        spans[k] = q0 + pass_q - qlos[k]
                        scs[k] = scpool.tile(
                            [128, pass_q], f32, tag="sc", name=f"sc_{h}_{p}_{k}"
                        )
                    # interleave even/odd chunks so the two row-groups of the
                    # PE array (d=64 contraction) run concurrently
                    chunk_lists = {k: _chunks(0, spans[k]) for k in pair}
                    n_ch = max(len(v) for v in chunk_lists.values())
                    for ci in range(n_ch):
                        for k in pair:
                            if ci >= len(chunk_lists[k]):
                                continue
                            c, w = chunk_lists[k][ci]
                            half = k % 2
                            nc.tensor.matmul(
                                scs[k][:, c : c + w],
                                kt2_v[64 * half : 64 * half + 64, k // 2],
                                qt2[64 * half : 64 * half + 64, qlos[k] + c : qlos[k] + c + w],
                                start=True,
                                stop=True,
                                skip_group_check=True,
                            )
                    if kp >= 2:
                        _flush_norm()
                    cur = []
                    for k in pair:
                        span = spans[k]
                        at = atpool.tile([128, pass_q], f16)
                        if ASSIGN[(p, k)] == "S":
                            nc.scalar.activation(
                                at[:, 0:span], scs[k][:, 0:span], Exp, scale=0.125
                            )
                        else:
                            nc.vector.tensor_scalar(
                                at[:, 0:span].bitcast(i16),
                                scs[k][:, 0:span],
                                SCH_A,
                                SCH_B,
                                Mult,
                                Add,
                            )
                        if 128 * k >= q0:
                            # zero the masked upper part of the diagonal block
                            nc.vector.tensor_mul(at[:, 0:128], at[:, 0:128], c_tri[:])
                        cur.append((k, at, qlos[k]))
                    pv_queue.append(cur)
                    if len(pv_queue) > 2:
                        _emit_pv(pv_queue.pop(0))
                for entries in pv_queue:
                    _emit_pv(entries)

                # evacuate PSUM early (split between ScalarE and VectorE) so
                # acc can be reused; the rest of the epilogue is deferred so
                # the next pass's QK matmuls hide it.
                osb = osbpool.tile([65, pass_q], f32, name=f"osb_{h}_{q0}", tag="osb")
                nc.scalar.copy(osb[:, 0:512], acc[0:65, 0:512])
                nc.vector.tensor_copy(osb[:, 512:1024], acc[0:65, 512:1024])

                def _norm(osb=osb, h=h, q0=q0):
                    # block j lives at col 65*j, except j=7 at 512: a matmul
                    # (transpose) output may not cross the PSUM bank boundary.
                    tr = trpool.tile([128, 577], f32, name=f"tr_{h}_{q0}", tag="tr")
                    tr_v = tr[:, 0:455].rearrange("p (t c) -> p t c", c=65)  # blocks 0..6
                    for j in range(n_qt):
                        off = 65 * j if j < 7 else 512
                        nc.tensor.transpose(
                            tr[:, off : off + 65],
                            osb[:, 128 * j : 128 * (j + 1)],
                            iden65[:],
                        )
                    rcol = nrmpool.tile([128, n_qt], f32, tag="rcol", name=f"rcol_{h}_{q0}")
                    nc.vector.tensor_copy(rcol[:, 0:7], tr_v[:, 0:7, 64:65])
                    nc.vector.tensor_copy(rcol[:, 7:8], tr[:, 576:577])
                    rcp = nrmpool.tile([128, n_qt], f32, tag="rcp", name=f"rcp_{h}_{q0}")
                    rsc = nrmpool.tile([128, n_qt], f32, tag="rsc", name=f"rsc_{h}_{q0}")
                    nc.vector.reciprocal_approx_accurate(rcp[:], rcol[:], rsc[:])
                    of = ofpool.tile([128, n_qt * 64], f32, tag="of", name=f"of_{h}_{q0}")
                    of_v = of.rearrange("p (t c) -> p t c", c=64)
                    rcp_b7 = rcp[:, 0:7].unsqueeze(2).broadcast_to([128, 7, 64])
                    nc.vector.tensor_mul(of_v[:, 0:7], tr_v[:, 0:7, 0:64], rcp_b7)
                    rcp_b1 = rcp[:, 7:8].broadcast_to([128, 64])
                    nc.vector.tensor_mul(of_v[:, 7], tr[:, 512:576], rcp_b1)
                    nc.sync.dma_start(
                        ot_d[h, q0 : q0 + pass_q].rearrange("(t p) d -> p t d", p=128),
                        of_v[:],
                    )

                pending_norm[0] = _norm
            _flush_norm()
            head_tiles = next_tiles[0]


def _make_consts():
    kk, qq = np.meshgrid(np.arange(128), np.arange(128), indexing="ij")
    tri = (kk <= qq).astype(np.float16)  # keep-mask for the diagonal block
    iden65 = np.eye(65, dtype=np.float32)
    return tri, iden65


_NC_CACHE = {}


def _build_nc(n_heads=HEADS_PER_CORE, s=S, pass_q=PASS_Q):
    key = (n_heads, s, pass_q)
    if key in _NC_CACHE:
        return _NC_CACHE[key]
    import concourse.tile as tile
    from concourse import bacc, mybir

    nc = bacc.Bacc(
        "TRN2", target_bir_lowering=False, debug=False, enable_asserts=False
    )
    f32 = mybir.dt.float32
    f16 = mybir.dt.float16
    ins = {
        "qt": nc.dram_tensor("qt", [n_heads, D, s], f16, kind="ExternalInput").ap(),
        "kt": nc.dram_tensor("kt", [n_heads, D, s], f16, kind="ExternalInput").ap(),
        "v": nc.dram_tensor("v", [n_heads, s, D + 1], f16, kind="ExternalInput").ap(),
        "ctri": nc.dram_tensor("ctri", [128, 128], f16, kind="ExternalInput").ap(),
        "ciden65": nc.dram_tensor("ciden65", [65, 65], f32, kind="ExternalInput").ap(),
    }
    outs = {
        "ot": nc.dram_tensor("ot", [n_heads, s, D], f32, kind="ExternalOutput").ap(),
    }
    with tile.TileContext(nc) as tc:
        build_attention(tc, outs, ins, n_heads=n_heads, s=s, pass_q=pass_q)
    nc.compile()
    _NC_CACHE[key] = nc
    return nc


def kernel(Q, K, V, mask, trace=False):
    """Full-input entry point: shards over 8 NeuronCores, returns full output."""
    from concourse.bass_utils import run_bass_kernel_spmd

    nc = _build_nc()
    tri, iden65 = _make_consts()

    Qf = np.ascontiguousarray(
        Q.reshape(B * H, S, D).transpose(0, 2, 1), dtype=np.float16
    )
    Kf = np.ascontiguousarray(
        K.reshape(B * H, S, D).transpose(0, 2, 1), dtype=np.float16
    )
    Vf = np.concatenate(
        [
            V.reshape(B * H, S, D).astype(np.float16),
            np.ones((B * H, S, 1), dtype=np.float16),
        ],
        axis=-1,
    )

    in_maps = []
    for c in range(N_CORES):
        sl = slice(c * HEADS_PER_CORE, (c + 1) * HEADS_PER_CORE)
        in_maps.append(
            {
                "qt": Qf[sl],
                "kt": Kf[sl],
                "v": Vf[sl],
                "ctri": tri,
                "ciden65": iden65,
            }
        )

    res = run_bass_kernel_spmd(nc, in_maps, core_ids=list(range(N_CORES)), trace=trace)
    ot = np.concatenate([res.results[c]["ot"] for c in range(N_CORES)], axis=0)
    out = ot.reshape(B, H, S, D)
    kernel.last_results = res
    return np.ascontiguousarray(out, dtype=np.float32)
